# revision 3
# baseline (speedup 1.0000x reference)
"""Trainium2 Bass kernel for nn_AttentionHead (B=8, S=2048, H=1024, D=64).

Strategy: data-parallel over batch -- one batch element per NeuronCore,
8 cores, no collectives.  Per core, attention is computed in "transposed
space" so that no large on-device transposes of activations are needed:

  - host passes query/key/value pre-transposed as [H, S] and the relative
    bias pre-transposed as [Sk, Sq] (cheap strided numpy copies);
  - projections on PE produce qT/kT/vT [64, S] directly;
  - scoresT[sk, sq] = kT.T-slice @ qT  (contraction over d=64 on partitions);
  - the relative bias is accumulated into the scores PSUM with an
    identity-weight matmul (no DVE pass);
  - exp on the scalar engine, no max-subtraction (logits are ~N(0,1),
    mathematically equivalent, overflow-impossible in f32);
  - softmax denominator comes for free from a ones-column appended to V
    (AV matmul also contracts the ones row -> row sums);
  - the {0,1} key mask is folded multiplicatively into V rows (+ ones col),
    exactly reproducing masked_fill(-inf) semantics;
  - out.T [65, S] accumulates in PSUM over sk; final PE transpose back to
    [S, 65], divide by the denominator column, DMA out.
"""

import os
from contextlib import ExitStack

import numpy as np

import concourse.bass as bass
import concourse.tile as tile
from concourse import bacc, mybir
from concourse.bass_utils import run_bass_kernel_spmd
from concourse.masks import make_identity

B, S, H, D = 8, 2048, 1024, 64
N_CORES = 8
FP = mybir.dt.float32

# compute dtype for matmul operands / bias / att probabilities
USE_BF16 = os.environ.get("KERNEL_BF16", "1") == "1"
CD = mybir.dt.bfloat16 if USE_BF16 else mybir.dt.float32

SQ_BLK = 1024  # sq columns processed per outer block (2 PSUM banks)
NT = S // SQ_BLK
NK = S // 128  # sk tiles
NH = H // 128  # hidden chunks


def _np_cd():
    if USE_BF16:
        import ml_dtypes

        return ml_dtypes.bfloat16
    return np.float32


def build_bass():
    nc = bacc.Bacc("TRN2", target_bir_lowering=False, debug=False,
                   num_devices=N_CORES)

    xqT = nc.dram_tensor("xqT", [H, S], CD, kind="ExternalInput").ap()
    xkT = nc.dram_tensor("xkT", [H, S], CD, kind="ExternalInput").ap()
    xvT = nc.dram_tensor("xvT", [H, S], CD, kind="ExternalInput").ap()
    biasT = nc.dram_tensor("biasT", [S, S], CD, kind="ExternalInput").ap()
    maskT = nc.dram_tensor("maskT", [128, NK], CD, kind="ExternalInput").ap()
    wqT = nc.dram_tensor("wqT", [H, D], CD, kind="ExternalInput").ap()
    wkT = nc.dram_tensor("wkT", [H, D], CD, kind="ExternalInput").ap()
    wvT = nc.dram_tensor("wvT", [H, D], CD, kind="ExternalInput").ap()
    bq = nc.dram_tensor("bq", [D, 1], FP, kind="ExternalInput").ap()
    bk = nc.dram_tensor("bk", [D, 1], FP, kind="ExternalInput").ap()
    bv = nc.dram_tensor("bv", [D, 1], FP, kind="ExternalInput").ap()
    out_d = nc.dram_tensor("out", [S, D], FP, kind="ExternalOutput").ap()

    with tile.TileContext(nc) as tc, ExitStack() as ctx:
        const = ctx.enter_context(tc.tile_pool(name="const", bufs=1))

        ident = const.tile([128, 128], FP, tag="ident")
        make_identity(nc, ident)
        if USE_BF16:
            ident_c = const.tile([128, 128], CD, tag="ident_c")
            nc.vector.tensor_copy(ident_c, ident)
        else:
            ident_c = ident

        # weights [128, NH, D]: element (p, t, d) = W.T[t*128+p, d]
        w_sb = {}
        for name, wT in (("q", wqT), ("k", wkT), ("v", wvT)):
            w = const.tile([128, NH, D], CD, tag=f"w{name}")
            nc.sync.dma_start(out=w, in_=wT.rearrange("(t p) d -> p t d", p=128))
            w_sb[name] = w
        b_sb = {}
        for name, bT in (("q", bq), ("k", bk), ("v", bv)):
            b = const.tile([D, 1], FP, tag=f"b{name}")
            nc.sync.dma_start(out=b, in_=bT)
            b_sb[name] = b
        mask_sb = const.tile([128, NK], CD, tag="mask")
        nc.sync.dma_start(out=mask_sb, in_=maskT)

        # persistent projected tensors
        qT_sb = const.tile([D, S], CD, tag="qT")
        kT_sb = const.tile([D, S], CD, tag="kT")
        vT_sb = const.tile([D, S], FP, tag="vT")
        v_aug = const.tile([128, NK, D + 1], CD, tag="v_aug")

        # ---- Phase P: projections ----
        with tc.tile_pool(name="xin", bufs=3) as xin, \
             tc.tile_pool(name="proj_ps", bufs=4, space="PSUM") as proj_ps, \
             tc.tile_pool(name="vt_ps", bufs=2, space="PSUM") as vt_ps_pool:
            for name, xT, dst in (("q", xqT, qT_sb), ("k", xkT, kT_sb),
                                  ("v", xvT, vT_sb)):
                ps = [proj_ps.tile([D, 512], FP, tag="proj", name=f"proj_{name}_{n}")
                      for n in range(4)]
                for h in range(NH):
                    x_sb = xin.tile([128, S], CD, tag="x")
                    nc.sync.dma_start(out=x_sb, in_=xT[h * 128:(h + 1) * 128, :])
                    for n in range(4):
                        nc.tensor.matmul(
                            ps[n], lhsT=w_sb[name][:, h, :],
                            rhs=x_sb[:, n * 512:(n + 1) * 512],
                            start=(h == 0), stop=(h == NH - 1))
                for n in range(4):
                    nc.scalar.activation(
                        out=dst[:, n * 512:(n + 1) * 512], in_=ps[n],
                        func=mybir.ActivationFunctionType.Identity,
                        bias=b_sb[name], scale=1.0)

            # v_aug[p, sk, :D] = vT.T rows scaled by mask; col D = mask
            for sk in range(NK):
                vt = vt_ps_pool.tile([128, D], FP, tag="vt")
                nc.tensor.matmul(vt, lhsT=vT_sb[:, sk * 128:(sk + 1) * 128],
                                 rhs=ident[:D, :D], is_transpose=True)
                nc.vector.tensor_scalar_mul(
                    out=v_aug[:, sk, 0:D], in0=vt,
                    scalar1=mask_sb[:, sk:sk + 1])
                nc.vector.tensor_copy(out=v_aug[:, sk, D:D + 1],
                                      in_=mask_sb[:, sk:sk + 1])

        # ---- Phase S: attention ----
        with tc.tile_pool(name="bias_in", bufs=4) as bias_in, \
             tc.tile_pool(name="att", bufs=3) as att_pool, \
             tc.tile_pool(name="avsb", bufs=2) as avsb_pool, \
             tc.tile_pool(name="fin", bufs=3) as fin_pool, \
             tc.tile_pool(name="sc_ps", bufs=2, space="PSUM") as sc_ps, \
             tc.tile_pool(name="av_ps", bufs=1, space="PSUM") as av_ps, \
             tc.tile_pool(name="ot_ps", bufs=2, space="PSUM") as ot_ps:
            for nt in range(NT):
                sq0 = nt * SQ_BLK
                av = av_ps.tile([D + 1, SQ_BLK], FP, tag="av")
                for sk in range(NK):
                    bias_t = bias_in.tile([128, SQ_BLK], CD, tag="bias")
                    nc.sync.dma_start(
                        out=bias_t,
                        in_=biasT[sk * 128:(sk + 1) * 128, sq0:sq0 + SQ_BLK])
                    sc = sc_ps.tile([128, SQ_BLK], FP, tag="sc")
                    for i in range(SQ_BLK // 512):
                        cols = slice(i * 512, (i + 1) * 512)
                        nc.tensor.matmul(
                            sc[:, cols],
                            lhsT=kT_sb[:, sk * 128:(sk + 1) * 128],
                            rhs=qT_sb[:, sq0 + i * 512:sq0 + (i + 1) * 512],
                            start=True, stop=False)
                        nc.tensor.matmul(sc[:, cols], lhsT=ident_c,
                                         rhs=bias_t[:, cols],
                                         start=False, stop=True)
                    att = att_pool.tile([128, SQ_BLK], CD, tag="att")
                    nc.scalar.activation(out=att, in_=sc,
                                         func=mybir.ActivationFunctionType.Exp,
                                         scale=1.0 / np.sqrt(float(D)))
                    for i in range(SQ_BLK // 512):
                        cols = slice(i * 512, (i + 1) * 512)
                        nc.tensor.matmul(av[:, cols], lhsT=v_aug[:, sk, :],
                                         rhs=att[:, cols],
                                         start=(sk == 0), stop=(sk == NK - 1))
                avs = avsb_pool.tile([D + 1, SQ_BLK], FP, tag="avs")
                nc.vector.tensor_copy(out=avs, in_=av)
                for j in range(SQ_BLK // 128):
                    ot = ot_ps.tile([128, D + 1], FP, tag="ot")
                    nc.tensor.matmul(ot, lhsT=avs[:, j * 128:(j + 1) * 128],
                                     rhs=ident[:D + 1, :D + 1],
                                     is_transpose=True)
                    rec = fin_pool.tile([128, 1], FP, tag="rec")
                    nc.vector.reciprocal(out=rec, in_=ot[:, D:D + 1])
                    fin = fin_pool.tile([128, D], FP, tag="fin")
                    nc.vector.tensor_scalar_mul(out=fin, in0=ot[:, 0:D],
                                                scalar1=rec)
                    r0 = sq0 + j * 128
                    nc.sync.dma_start(out=out_d[r0:r0 + 128, :], in_=fin)

    nc.compile()
    return nc


_NC = None


def _get_nc():
    global _NC
    if _NC is None:
        _NC = build_bass()
    return _NC


def _prep_core_inputs(b, query, key, value, relative_biases, mask,
                      Wq, bq, Wk, bk, Wv, bv):
    cd = _np_cd()
    return {
        "xqT": np.ascontiguousarray(query[b].T.astype(cd, copy=False)),
        "xkT": np.ascontiguousarray(key[b].T.astype(cd, copy=False)),
        "xvT": np.ascontiguousarray(value[b].T.astype(cd, copy=False)),
        "biasT": np.ascontiguousarray(
            relative_biases[b].T.astype(cd, copy=False)),
        "maskT": np.ascontiguousarray(
            mask[b].astype(cd).reshape(NK, 128).T),
        "wqT": np.ascontiguousarray(Wq.T.astype(cd, copy=False)),
        "wkT": np.ascontiguousarray(Wk.T.astype(cd, copy=False)),
        "wvT": np.ascontiguousarray(Wv.T.astype(cd, copy=False)),
        "bq": np.asarray(bq, np.float32).reshape(D, 1),
        "bk": np.asarray(bk, np.float32).reshape(D, 1),
        "bv": np.asarray(bv, np.float32).reshape(D, 1),
    }


def kernel(query, key, value, relative_biases, mask, Wq, bq, Wk, bk, Wv, bv):
    query = np.asarray(query, np.float32)
    key = np.asarray(key, np.float32)
    value = np.asarray(value, np.float32)
    relative_biases = np.asarray(relative_biases, np.float32)
    mask = np.asarray(mask)
    Wq, Wk, Wv = (np.asarray(w, np.float32) for w in (Wq, Wk, Wv))

    nc = _get_nc()
    in_maps = [
        _prep_core_inputs(b, query, key, value, relative_biases, mask,
                          Wq, bq, Wk, bk, Wv, bv)
        for b in range(B)
    ]
    res = run_bass_kernel_spmd(nc, in_maps, core_ids=list(range(N_CORES)))
    out = np.stack([res.results[i]["out"] for i in range(N_CORES)], axis=0)
    return out.astype(np.float32)


# revision 4
# speedup vs baseline: 2.5097x; 2.5097x over previous
"""Trainium2 Bass kernel for nn_AttentionHead (B=8, S=2048, H=1024, D=64).

Strategy: data-parallel over batch -- one batch element per NeuronCore,
8 cores, no collectives.  Per core, attention is computed in "transposed
space" so that no large on-device transposes of activations are needed:

  - host passes query/key/value pre-transposed as [H, S] and the relative
    bias pre-transposed as [Sk, Sq] (cheap strided numpy copies);
  - projections on PE produce qT/kT/vT [64, S] directly;
  - scoresT[sk, sq] = kT.T-slice @ qT  (contraction over d=64 on partitions);
  - the relative bias is accumulated into the scores PSUM with an
    identity-weight matmul (no DVE pass);
  - exp on the scalar engine, no max-subtraction (logits are ~N(0,1),
    mathematically equivalent, overflow-impossible in f32);
  - softmax denominator comes for free from a ones-column appended to V
    (AV matmul also contracts the ones row -> row sums);
  - the {0,1} key mask is folded multiplicatively into V rows (+ ones col),
    exactly reproducing masked_fill(-inf) semantics;
  - out.T [65, S] accumulates in PSUM over sk; final PE transpose back to
    [S, 65], divide by the denominator column, DMA out.
"""

import os
from contextlib import ExitStack

import numpy as np

import concourse.bass as bass
import concourse.tile as tile
from concourse import bacc, mybir
from concourse.bass_utils import run_bass_kernel_spmd
from concourse.masks import make_identity

B, S, H, D = 8, 2048, 1024, 64
N_CORES = 8
FP = mybir.dt.float32

# compute dtype for matmul operands / bias / att probabilities
USE_BF16 = os.environ.get("KERNEL_BF16", "1") == "1"
CD = mybir.dt.bfloat16 if USE_BF16 else mybir.dt.float32

SQ_BLK = 1024  # sq columns processed per outer block (2 PSUM banks)
NT = S // SQ_BLK
NK = S // 128  # sk tiles
NH = H // 128  # hidden chunks


def _np_cd():
    if USE_BF16:
        import ml_dtypes

        return ml_dtypes.bfloat16
    return np.float32


def build_bass():
    nc = bacc.Bacc("TRN2", target_bir_lowering=False, debug=False,
                   num_devices=N_CORES)

    xqT = nc.dram_tensor("xqT", [H, S], CD, kind="ExternalInput").ap()
    xkT = nc.dram_tensor("xkT", [H, S], CD, kind="ExternalInput").ap()
    xvT = nc.dram_tensor("xvT", [H, S], CD, kind="ExternalInput").ap()
    biasT = nc.dram_tensor("biasT", [S, S], CD, kind="ExternalInput").ap()
    maskT = nc.dram_tensor("maskT", [128, NK], FP, kind="ExternalInput").ap()
    wqT = nc.dram_tensor("wqT", [H, D], CD, kind="ExternalInput").ap()
    wkT = nc.dram_tensor("wkT", [H, D], CD, kind="ExternalInput").ap()
    wvT = nc.dram_tensor("wvT", [H, D], CD, kind="ExternalInput").ap()
    bq = nc.dram_tensor("bq", [D, 1], FP, kind="ExternalInput").ap()
    bk = nc.dram_tensor("bk", [D, 1], FP, kind="ExternalInput").ap()
    bv = nc.dram_tensor("bv", [D, 1], FP, kind="ExternalInput").ap()
    out_d = nc.dram_tensor("out", [S, D], FP, kind="ExternalOutput").ap()

    with tile.TileContext(nc) as tc, ExitStack() as ctx:
        const = ctx.enter_context(tc.tile_pool(name="const", bufs=1))

        ident = const.tile([128, 128], FP, tag="ident")
        make_identity(nc, ident)
        if USE_BF16:
            ident_c = const.tile([128, 128], CD, tag="ident_c")
            nc.vector.tensor_copy(ident_c, ident)
        else:
            ident_c = ident

        # weights [128, NH, D]: element (p, t, d) = W.T[t*128+p, d]
        w_sb = {}
        for name, wT in (("q", wqT), ("k", wkT), ("v", wvT)):
            w = const.tile([128, NH, D], CD, tag=f"w{name}")
            nc.sync.dma_start(out=w, in_=wT.rearrange("(t p) d -> p t d", p=128))
            w_sb[name] = w
        b_sb = {}
        for name, bT in (("q", bq), ("k", bk), ("v", bv)):
            b = const.tile([D, 1], FP, tag=f"b{name}")
            nc.sync.dma_start(out=b, in_=bT)
            b_sb[name] = b
        mask_sb = const.tile([128, NK], FP, tag="mask")
        nc.sync.dma_start(out=mask_sb, in_=maskT)

        # persistent projected tensors
        qT_sb = const.tile([D, S], CD, tag="qT")
        kT_sb = const.tile([D, S], CD, tag="kT")
        vT_sb = const.tile([D, S], FP, tag="vT")
        v_aug = const.tile([128, NK, D + 1], CD, tag="v_aug")

        # ---- Phase P: projections ----
        with tc.tile_pool(name="xin", bufs=3) as xin, \
             tc.tile_pool(name="proj_ps", bufs=4, space="PSUM") as proj_ps, \
             tc.tile_pool(name="vt_ps", bufs=2, space="PSUM") as vt_ps_pool:
            for name, xT, dst in (("q", xqT, qT_sb), ("k", xkT, kT_sb),
                                  ("v", xvT, vT_sb)):
                ps = [proj_ps.tile([D, 512], FP, tag="proj", name=f"proj_{name}_{n}")
                      for n in range(4)]
                for h in range(NH):
                    x_sb = xin.tile([128, S], CD, tag="x")
                    nc.sync.dma_start(out=x_sb, in_=xT[h * 128:(h + 1) * 128, :])
                    for n in range(4):
                        nc.tensor.matmul(
                            ps[n], lhsT=w_sb[name][:, h, :],
                            rhs=x_sb[:, n * 512:(n + 1) * 512],
                            start=(h == 0), stop=(h == NH - 1))
                for n in range(4):
                    nc.scalar.activation(
                        out=dst[:, n * 512:(n + 1) * 512], in_=ps[n],
                        func=mybir.ActivationFunctionType.Identity,
                        bias=b_sb[name], scale=1.0)

            # v_aug[p, sk, :D] = vT.T rows scaled by mask; col D = mask
            for sk in range(NK):
                vt = vt_ps_pool.tile([128, D], FP, tag="vt")
                nc.tensor.matmul(vt, lhsT=vT_sb[:, sk * 128:(sk + 1) * 128],
                                 rhs=ident[:D, :D], is_transpose=True)
                nc.vector.tensor_scalar_mul(
                    out=v_aug[:, sk, 0:D], in0=vt,
                    scalar1=mask_sb[:, sk:sk + 1])
                nc.vector.tensor_copy(out=v_aug[:, sk, D:D + 1],
                                      in_=mask_sb[:, sk:sk + 1])

        # ---- Phase S: attention ----
        with tc.tile_pool(name="bias_in", bufs=4) as bias_in, \
             tc.tile_pool(name="att", bufs=3) as att_pool, \
             tc.tile_pool(name="avsb", bufs=2) as avsb_pool, \
             tc.tile_pool(name="fin", bufs=3) as fin_pool, \
             tc.tile_pool(name="sc_ps", bufs=2, space="PSUM") as sc_ps, \
             tc.tile_pool(name="av_ps", bufs=1, space="PSUM") as av_ps, \
             tc.tile_pool(name="ot_ps", bufs=2, space="PSUM") as ot_ps:
            for nt in range(NT):
                sq0 = nt * SQ_BLK
                av = av_ps.tile([D + 1, SQ_BLK], FP, tag="av")
                for sk in range(NK):
                    bias_t = bias_in.tile([128, SQ_BLK], CD, tag="bias")
                    nc.sync.dma_start(
                        out=bias_t,
                        in_=biasT[sk * 128:(sk + 1) * 128, sq0:sq0 + SQ_BLK])
                    sc = sc_ps.tile([128, SQ_BLK], FP, tag="sc")
                    for i in range(SQ_BLK // 512):
                        cols = slice(i * 512, (i + 1) * 512)
                        nc.tensor.matmul(
                            sc[:, cols],
                            lhsT=kT_sb[:, sk * 128:(sk + 1) * 128],
                            rhs=qT_sb[:, sq0 + i * 512:sq0 + (i + 1) * 512],
                            start=True, stop=False)
                        nc.tensor.matmul(sc[:, cols], lhsT=ident_c,
                                         rhs=bias_t[:, cols],
                                         start=False, stop=True)
                    att = att_pool.tile([128, SQ_BLK], CD, tag="att")
                    nc.scalar.activation(out=att, in_=sc,
                                         func=mybir.ActivationFunctionType.Exp,
                                         scale=1.0 / np.sqrt(float(D)))
                    for i in range(SQ_BLK // 512):
                        cols = slice(i * 512, (i + 1) * 512)
                        nc.tensor.matmul(av[:, cols], lhsT=v_aug[:, sk, :],
                                         rhs=att[:, cols],
                                         start=(sk == 0), stop=(sk == NK - 1))
                avs = avsb_pool.tile([D + 1, SQ_BLK], FP, tag="avs")
                nc.vector.tensor_copy(out=avs, in_=av)
                for j in range(SQ_BLK // 128):
                    ot = ot_ps.tile([128, D + 1], FP, tag="ot")
                    nc.tensor.matmul(ot, lhsT=avs[:, j * 128:(j + 1) * 128],
                                     rhs=ident[:D + 1, :D + 1],
                                     is_transpose=True)
                    rec = fin_pool.tile([128, 1], FP, tag="rec")
                    nc.vector.reciprocal(out=rec, in_=ot[:, D:D + 1])
                    fin = fin_pool.tile([128, D], FP, tag="fin")
                    nc.vector.tensor_scalar_mul(out=fin, in0=ot[:, 0:D],
                                                scalar1=rec)
                    r0 = sq0 + j * 128
                    nc.sync.dma_start(out=out_d[r0:r0 + 128, :], in_=fin)

    nc.compile()
    return nc


_NC = None


def _get_nc():
    global _NC
    if _NC is None:
        _NC = build_bass()
    return _NC


def _prep_core_inputs(b, query, key, value, relative_biases, mask,
                      Wq, bq, Wk, bk, Wv, bv):
    cd = _np_cd()
    return {
        "xqT": np.ascontiguousarray(query[b].T.astype(cd, copy=False)),
        "xkT": np.ascontiguousarray(key[b].T.astype(cd, copy=False)),
        "xvT": np.ascontiguousarray(value[b].T.astype(cd, copy=False)),
        "biasT": np.ascontiguousarray(
            relative_biases[b].T.astype(cd, copy=False)),
        "maskT": np.ascontiguousarray(
            mask[b].astype(np.float32).reshape(NK, 128).T),
        "wqT": np.ascontiguousarray(Wq.T.astype(cd, copy=False)),
        "wkT": np.ascontiguousarray(Wk.T.astype(cd, copy=False)),
        "wvT": np.ascontiguousarray(Wv.T.astype(cd, copy=False)),
        "bq": np.asarray(bq, np.float32).reshape(D, 1),
        "bk": np.asarray(bk, np.float32).reshape(D, 1),
        "bv": np.asarray(bv, np.float32).reshape(D, 1),
    }


def kernel(query, key, value, relative_biases, mask, Wq, bq, Wk, bk, Wv, bv):
    query = np.asarray(query, np.float32)
    key = np.asarray(key, np.float32)
    value = np.asarray(value, np.float32)
    relative_biases = np.asarray(relative_biases, np.float32)
    mask = np.asarray(mask)
    Wq, Wk, Wv = (np.asarray(w, np.float32) for w in (Wq, Wk, Wv))

    nc = _get_nc()
    in_maps = [
        _prep_core_inputs(b, query, key, value, relative_biases, mask,
                          Wq, bq, Wk, bk, Wv, bv)
        for b in range(B)
    ]
    res = run_bass_kernel_spmd(nc, in_maps, core_ids=list(range(N_CORES)))
    out = np.stack([res.results[i]["out"] for i in range(N_CORES)], axis=0)
    return out.astype(np.float32)
